# revision 12
# baseline (speedup 1.0000x reference)
"""FLAME layer on 8 Trainium2 NeuronCores (Bass/Tile).

Strategy (vertex-parallel, single-pass bf16):
  * V=5023 padded to 5120, 640 vertices/core; every core handles the full
    batch B=2048 for its vertex slice.  Model tensors sliced 1/8 per core,
    per-batch operands (betas, pose features, joint transforms) replicated.
  * Host: rodrigues, FK chain, A_rel; J = J0 + betas @ SJ (linear in betas).
  * Gate is rel_err < 2e-2, so everything runs one bf16 pass (no hi/lo
    splits): ~3x less PE + input DMA than the 3-pass baseline.
  * Device per 128-vertex chunk, [v(part), b(free)] layout:
      1. vposed_c = sdt_c.T @ betas_aug    (PE, K=437, fp32 PSUM)
         -> scalar-copies to SBUF bf16 (vp planes)
      2. T planes (c,n): K=5 matmuls, 4-way row-tiled via tile_position
         -> T_c1/T_c2 scalar-copied to SBUF bf16;
            T_c0/T_c3 consumed straight from PSUM by DVE (1x reads)
      3. apply: m0 = T_c0(PSUM)*vp0 (DVE), m1 = T_c1*vp1 (DVE bf16 2x),
         m2 = T_c2*vp2 (DVE), b = m2 + T_c3(PSUM) (DVE),
         a = m0 + m1 (GpSimd), out = a + b rides the output DMA
         (SWDGE accumulate) in bf16.
      4. host converts bf16 -> f32 and reassembles [B, V, 3].
"""

import os
from contextlib import ExitStack

import ml_dtypes
import numpy as np

import bass_rust
import concourse.bass as bass
import concourse.mybir as mybir
import concourse.tile as tile_mod
from concourse.bass_utils import run_bass_kernel_spmd

# ---------------------------------------------------------------- constants
B = 2048
V = 5023
VP = 5120            # padded vertex count (8 cores x 640)
NVC = VP // 8        # vertices per core
NCHUNK = NVC // 128  # 128-vertex chunks per core (5)
NJ = 5
NCORES = 8
BH = 1024            # batch half (free-dim cell for T/apply)
KB = 437             # vposed contraction rows: 400 betas + 1 ones + 36 pose
KCH = [(k, min(128, KB - k)) for k in range(0, KB, 128)]
NS = 512             # matmul free-dim slice (one PSUM bank)
PARENTS = np.array([-1, 0, 1, 1, 1])

BF16 = ml_dtypes.bfloat16

# ------------------------------------------------- walrus multi-wait patch
# This walrus build accepts only ONE sem-wait per instruction (CTRL and
# LW queue structs alike), but Tile freely assigns several. Split the
# surplus waits onto same-engine NOPs emitted immediately before the
# instruction — the engine stalls on each NOP's wait first, so the
# gating semantics are identical.


def _patched_commit_instruction(self, inst, lazy_reg_writes=True):
    si = inst.sync_info
    if si is not None and len(si.on_wait) > 1:
        waits = list(si.on_wait)
        inst.sync_info = bass_rust.SyncInfo(
            on_update=list(si.on_update), on_wait=waits[:1]
        )
        for w in waits[1:]:
            nop = mybir.InstNoOp(
                name=self.nc.get_next_instruction_name(),
                engine=inst.engine,
                ins=[],
                outs=[],
                bass_nofuse=True,
                sync_info=bass_rust.SyncInfo(on_update=[], on_wait=[w]),
            )
            _orig_commit_instruction(self, nop, lazy_reg_writes=False)
    return _orig_commit_instruction(self, inst, lazy_reg_writes)


def _split_inst_waits(nc, inst):
    si = inst.ins.sync_info
    if si is None:
        return
    waits = list(si.on_wait)
    if len(waits) <= 1:
        return
    inst.ins.sync_info = bass_rust.SyncInfo(
        on_update=list(si.on_update), on_wait=waits[:1]
    )
    for i in range(1, len(waits)):
        nop = nc.sync.nop(nofuse=True, hint="drain_wait_split")
        nop.ins.sync_info = bass_rust.SyncInfo(on_update=[], on_wait=[waits[i]])


def _patched_drain_and_barrier(self, tick_clock, wait_clock):
    nc = self.nc
    drain_inst = nc.sync.drain()
    wait_clock.add_sem_waits(
        drain_inst.ins, tile_mod.ScopedClock({None: tick_clock.global_clock})
    )
    _split_inst_waits(nc, drain_inst)
    nc.all_engine_barrier()
    assert self.sems is not None
    popped = nc._tile_sem_poison_stack.pop()
    assert popped is self._sem_poison
    nc.clear_and_free_semaphores(list(self.sems.allocated().values()))
    nc.all_engine_barrier()


_orig_commit_instruction = tile_mod.TileContext._commit_instruction
if getattr(tile_mod.TileContext, "_flame_wait_patch", False) is False:
    tile_mod.TileContext._commit_instruction = _patched_commit_instruction
    tile_mod.TileContext._drain_and_barrier = _patched_drain_and_barrier
    tile_mod.TileContext._flame_wait_patch = True

# ----------------------------------------------------------- host-side math


def _rodrigues(r):
    angle = np.linalg.norm(r, axis=-1, keepdims=True) + 1e-8
    axis = r / angle
    x, y, z = axis[..., 0], axis[..., 1], axis[..., 2]
    zero = np.zeros_like(x)
    K = np.stack([zero, -z, y, z, zero, -x, -y, x, zero], axis=-1)
    K = K.reshape(r.shape[:-1] + (3, 3))
    s = np.sin(angle)[..., None]
    c = np.cos(angle)[..., None]
    return np.eye(3, dtype=r.dtype) + s * K + (1.0 - c) * (K @ K)


def _host_batch_prep(shape, expression, rotation, neck, jaw, eyeballs,
                     v_template, shapedirs, J_regressor):
    f64 = np.float64
    b = shape.shape[0]
    betas = np.concatenate([shape, expression], axis=1).astype(f64)
    full_pose = np.concatenate([rotation, neck, jaw, eyeballs], axis=1).astype(f64)

    jr = J_regressor.astype(f64)
    J0 = jr @ v_template.astype(f64)                                   # [5,3]
    SJ = np.einsum('jv,vcl->ljc', jr, shapedirs.astype(f64)).reshape(400, NJ * 3)
    J = (J0.reshape(-1) + betas @ SJ).reshape(b, NJ, 3)

    rot_mats = _rodrigues(full_pose.reshape(b, NJ, 3))
    pose_feature = (rot_mats[:, 1:] - np.eye(3, dtype=f64)).reshape(b, 36)

    rel_joints = np.concatenate([J[:, :1], J[:, 1:] - J[:, PARENTS[1:]]], axis=1)
    T_local = np.zeros((b, NJ, 4, 4), dtype=f64)
    T_local[:, :, :3, :3] = rot_mats
    T_local[:, :, :3, 3] = rel_joints
    T_local[:, :, 3, 3] = 1.0
    chain = [T_local[:, 0]]
    for j in range(1, NJ):
        chain.append(chain[PARENTS[j]] @ T_local[:, j])
    A = np.stack(chain, axis=1)

    j_hom = np.concatenate([J, np.zeros_like(J[..., :1])], axis=-1)
    t_corr = np.einsum('bjmn,bjn->bjm', A, j_hom)
    A_rel = (A - np.concatenate(
        [np.zeros_like(A[..., :3]), t_corr[..., None]], axis=-1)
    ).astype(np.float32)                                               # [B,5,4,4]

    # betas_aug [437, B]: rows 0-399 betas.T, 400 ones, 401-436 pose_feature.T
    betas_aug = np.empty((KB, b), dtype=np.float32)
    betas_aug[:400] = betas.T
    betas_aug[400] = 1.0
    betas_aug[401:] = pose_feature.T
    bt = betas_aug.astype(BF16)

    # artc[c, 32n+j, b] = A_rel[b, j, c, n]; rows at 32-offsets so the
    # K=5 T-blend matmuls can row-tile (tile_position=(32n, 0)).
    artc = np.zeros((3, 128, b), dtype=np.float32)
    for c in range(3):
        for n in range(4):
            for j in range(NJ):
                artc[c, 32 * n + j] = A_rel[:, j, c, n]
    artc = artc.astype(BF16)
    return bt, artc


def _host_model_prep(v_template, shapedirs, posedirs, lbs_weights):
    # sdt [3, 437, VP] matching betas_aug rows
    sdt = np.zeros((3, KB, VP), dtype=np.float32)
    sdt[:, :400, :V] = shapedirs.transpose(1, 2, 0)
    sdt[:, 400, :V] = v_template.T
    sdt[:, 401:, :V] = posedirs.reshape(36, V, 3).transpose(2, 0, 1)
    sdt = sdt.astype(BF16)

    # wrt [128, VP]: rows 32n+j = w[:, j] for every group n
    wrt = np.zeros((128, VP), dtype=np.float32)
    for n in range(4):
        for j in range(NJ):
            wrt[32 * n + j, :V] = lbs_weights[:, j]
    wrt = wrt.astype(BF16)
    return sdt, wrt

# ------------------------------------------------------------ device kernel


def _build_device_program():
    nc = bass.Bass("TRN2", target_bir_lowering=False, debug=False)
    f32 = mybir.dt.float32
    bf16 = mybir.dt.bfloat16

    sdt = nc.dram_tensor("sdt", [3, KB, NVC], bf16, kind="ExternalInput").ap()
    wrt = nc.dram_tensor("wrt", [128, NVC], bf16, kind="ExternalInput").ap()
    bt = nc.dram_tensor("bt", [KB, B], bf16, kind="ExternalInput").ap()
    artc = nc.dram_tensor("artc", [3, 128, B], bf16, kind="ExternalInput").ap()
    out = nc.dram_tensor("out", [3, NVC, B], bf16, kind="ExternalOutput").ap()

    with tile_mod.TileContext(nc) as tc, ExitStack() as ctx:
        cpool = ctx.enter_context(tc.tile_pool(name="const", bufs=1))
        spool = ctx.enter_context(tc.tile_pool(name="stream", bufs=3))
        vpool = ctx.enter_context(tc.tile_pool(name="vposed", bufs=2))
        tspool = ctx.enter_context(tc.tile_pool(name="tsb", bufs=3))
        apool = ctx.enter_context(tc.tile_pool(name="apply", bufs=3))
        # one shared PSUM pool: 4 bank-tags x bufs=2 = all 8 banks.
        # vposed accumulators round-robin through the tags; each T-round
        # grabs one generation of all 4 tags.
        ps = ctx.enter_context(tc.tile_pool(name="ps", bufs=2, space="PSUM"))
        ps_rr = [0]

        def ps_tile():
            t = ps.tile([128, NS], mybir.dt.float32, tag=f"ps{ps_rr[0]}",
                        name=f"psacc{ps_rr[0]}")
            ps_rr[0] = (ps_rr[0] + 1) % 4
            return t

        def ps_round():
            assert ps_rr[0] == 0
            return [ps.tile([128, NS], mybir.dt.float32, tag=f"ps{n}",
                            name=f"pst{n}")
                    for n in range(4)]

        # resident operands (scalar HWDGE queue so the per-chunk streaming
        # DMAs on the sync queue aren't stuck behind them at startup)
        btt = []
        for ki, (k0, kn) in enumerate(KCH):
            th = cpool.tile([kn, B], bf16, tag=f"bt{ki}", name=f"bt{ki}")
            for cs in range(0, B, NS):
                nc.scalar.dma_start(th[:, cs:cs + NS],
                                    bt[k0:k0 + kn, cs:cs + NS])
            btt.append(th)
        wrtt = cpool.tile([128, NVC], bf16, tag="wrtt")
        nc.scalar.dma_start(wrtt[:], wrt[:, :])
        artt = []
        for c in range(3):
            t = cpool.tile([128, B], bf16, tag=f"artc{c}", name=f"artc{c}")
            nc.scalar.dma_start(t[:], artc[c, :, :])
            artt.append(t)

        for k in range(NCHUNK):
            vs = slice(k * 128, (k + 1) * 128)

            # stream this chunk's vposed lhsT tiles
            st = []
            for c in range(3):
                row = []
                for ki, (k0, kn) in enumerate(KCH):
                    t = spool.tile([kn, 128], bf16, tag=f"st{c}_{ki}",
                                   name=f"st{c}_{ki}")
                    nc.sync.dma_start(t[:], sdt[c, k0:k0 + kn, vs])
                    row.append(t)
                st.append(row)

            # 1) vposed planes [128, B]: K=437 in 4 chunks, fp32 PSUM,
            #    scalar-copied to SBUF bf16
            nkc = len(KCH)
            vp = []
            for c in range(3):
                dst = vpool.tile([128, B], bf16, tag=f"vp{c}")
                for s in range(B // NS):
                    acc = ps_tile()
                    bs = slice(s * NS, (s + 1) * NS)
                    for ki in range(nkc):
                        nc.tensor.matmul(
                            acc[:], lhsT=st[c][ki][:],
                            rhs=btt[ki][:, bs],
                            start=(ki == 0), stop=(ki == nkc - 1))
                    nc.scalar.copy(out=dst[:, bs], in_=acc[:])
                vp.append(dst)

            # 2+3) per c: T planes + apply over the full batch.
            # Four independent per-plane pipelines, each owning one PSUM
            # bank tag (x2 generations) so chains stay 2 deep.
            def t_mm(dst, c, n, bs):
                p0 = 32 * n
                nc.tensor.matmul(
                    dst[:], lhsT=wrtt[p0:p0 + NJ, vs],
                    rhs=artt[c][p0:p0 + NJ, bs],
                    start=True, stop=True, tile_position=(p0, 0))

            for c in range(3):
                # scalar-copied T planes (bf16 SBUF, full batch)
                t1s = tspool.tile([128, B], bf16, tag="t1s")
                t2s = tspool.tile([128, B], bf16, tag="t2s")
                m0 = apool.tile([128, B], bf16, tag="m0")
                m1 = apool.tile([128, B], bf16, tag="m1")
                m2 = apool.tile([128, B], bf16, tag="m2")
                bpl = apool.tile([128, B], bf16, tag="b")

                for sub in range(B // NS):   # n=1,2 series: MM -> copy
                    bs = slice(sub * NS, (sub + 1) * NS)
                    tt1 = ps.tile([128, NS], f32, tag="ps1", name="tt1")
                    t_mm(tt1, c, 1, bs)
                    nc.scalar.copy(out=t1s[:, bs], in_=tt1[:])
                    tt2 = ps.tile([128, NS], f32, tag="ps2", name="tt2")
                    t_mm(tt2, c, 2, bs)
                    nc.scalar.copy(out=t2s[:, bs], in_=tt2[:])
                # fat SBUF-only muls over the full batch
                nc.vector.tensor_mul(m1[:], t1s[:], vp[1][:])
                if c == 1:
                    nc.gpsimd.tensor_mul(m2[:], t2s[:], vp[2][:])
                else:
                    nc.vector.tensor_mul(m2[:], t2s[:], vp[2][:])
                for sub in range(B // NS):   # n=0 series: MM -> psum mul
                    bs = slice(sub * NS, (sub + 1) * NS)
                    tt0 = ps.tile([128, NS], f32, tag="ps0", name="tt0")
                    t_mm(tt0, c, 0, bs)
                    nc.vector.tensor_mul(m0[:, bs], tt0[:], vp[0][:, bs])
                for sub in range(B // NS):   # n=3 series: MM -> psum add
                    bs = slice(sub * NS, (sub + 1) * NS)
                    tt3 = ps.tile([128, NS], f32, tag="ps3", name="tt3")
                    t_mm(tt3, c, 3, bs)
                    nc.vector.tensor_add(bpl[:, bs], m2[:, bs], tt3[:])
                # sums ride the output DMA (SWDGE accumulate; same gp
                # queue so write -> accum -> accum stays ordered)
                nc.gpsimd.dma_start(out[c, vs, :], m0[:])
                nc.gpsimd.dma_start(out[c, vs, :], m1[:],
                                    accum_op=mybir.AluOpType.add)
                nc.gpsimd.dma_start(out[c, vs, :], bpl[:],
                                    accum_op=mybir.AluOpType.add)
    return nc


_NC_CACHE = {}


def _get_nc():
    if "nc" not in _NC_CACHE:
        _NC_CACHE["nc"] = _build_device_program()
    return _NC_CACHE["nc"]

# ---------------------------------------------------------------- entry


def build_in_maps(shape, expression, rotation, neck, jaw, eyeballs,
                  v_template, shapedirs, posedirs, J_regressor, lbs_weights):
    bt, artc = _host_batch_prep(
        shape, expression, rotation, neck, jaw, eyeballs,
        v_template, shapedirs, J_regressor)
    sdt, wrt = _host_model_prep(v_template, shapedirs, posedirs, lbs_weights)

    in_maps = []
    for i in range(NCORES):
        v0, v1 = i * NVC, (i + 1) * NVC
        in_maps.append({
            "sdt": np.ascontiguousarray(sdt[:, :, v0:v1]),
            "wrt": np.ascontiguousarray(wrt[:, v0:v1]),
            "bt": bt,
            "artc": artc,
        })
    return in_maps


def kernel(shape, expression, rotation, neck, jaw, eyeballs,
           v_template, shapedirs, posedirs, J_regressor, lbs_weights):
    in_maps = build_in_maps(shape, expression, rotation, neck, jaw, eyeballs,
                            v_template, shapedirs, posedirs, J_regressor,
                            lbs_weights)
    nc = _get_nc()
    res = run_bass_kernel_spmd(nc, in_maps, core_ids=list(range(NCORES)))

    full = np.concatenate(
        [np.asarray(res.results[i]["out"]) for i in range(NCORES)], axis=1)
    verts = np.ascontiguousarray(
        full[:, :V, :].transpose(2, 1, 0)).astype(np.float32)
    return verts


# revision 17
# speedup vs baseline: 1.0260x; 1.0260x over previous
"""FLAME layer on 8 Trainium2 NeuronCores (Bass/Tile).

Strategy (vertex-parallel, single-pass bf16):
  * V=5023 padded to 5120, 640 vertices/core; every core handles the full
    batch B=2048 for its vertex slice.  Model tensors sliced 1/8 per core,
    per-batch operands (betas, pose features, joint transforms) replicated.
  * Host: rodrigues, FK chain, A_rel; J = J0 + betas @ SJ (linear in betas).
  * Gate is rel_err < 2e-2, so everything runs one bf16 pass (no hi/lo
    splits): ~3x less PE + input DMA than the 3-pass baseline.
  * Device per 128-vertex chunk, [v(part), b(free)] layout:
      1. vposed_c = sdt_c.T @ betas_aug    (PE, K=437, fp32 PSUM)
         -> scalar-copies to SBUF bf16 (vp planes)
      2. T planes (c,n): K=5 matmuls, 4-way row-tiled via tile_position
         -> T_c1/T_c2 scalar-copied to SBUF bf16;
            T_c0/T_c3 consumed straight from PSUM by DVE (1x reads)
      3. apply: m0 = T_c0(PSUM)*vp0 (DVE), m1 = T_c1*vp1 (DVE bf16 2x),
         m2 = T_c2*vp2 (DVE), b = m2 + T_c3(PSUM) (DVE),
         a = m0 + m1 (GpSimd), out = a + b rides the output DMA
         (SWDGE accumulate) in bf16.
      4. host converts bf16 -> f32 and reassembles [B, V, 3].
"""

import os
from contextlib import ExitStack

import ml_dtypes
import numpy as np

import bass_rust
import concourse.bass as bass
import concourse.mybir as mybir
import concourse.tile as tile_mod
from concourse.bass_utils import run_bass_kernel_spmd

# ---------------------------------------------------------------- constants
B = 2048
V = 5023
VP = 5120            # padded vertex count (8 cores x 640)
NVC = VP // 8        # vertices per core
NCHUNK = NVC // 128  # 128-vertex chunks per core (5)
NJ = 5
NCORES = 8
BH = 1024            # batch half (free-dim cell for T/apply)
KB = 437             # vposed contraction rows: 400 betas + 1 ones + 36 pose
KCH = [(k, min(128, KB - k)) for k in range(0, KB, 128)]
NS = 512             # matmul free-dim slice (one PSUM bank)
PARENTS = np.array([-1, 0, 1, 1, 1])

BF16 = ml_dtypes.bfloat16

# ------------------------------------------------- walrus multi-wait patch
# This walrus build accepts only ONE sem-wait per instruction (CTRL and
# LW queue structs alike), but Tile freely assigns several. Split the
# surplus waits onto same-engine NOPs emitted immediately before the
# instruction — the engine stalls on each NOP's wait first, so the
# gating semantics are identical.


def _patched_commit_instruction(self, inst, lazy_reg_writes=True):
    si = inst.sync_info
    if si is not None and len(si.on_wait) > 1:
        waits = list(si.on_wait)
        inst.sync_info = bass_rust.SyncInfo(
            on_update=list(si.on_update), on_wait=waits[:1]
        )
        for w in waits[1:]:
            nop = mybir.InstNoOp(
                name=self.nc.get_next_instruction_name(),
                engine=inst.engine,
                ins=[],
                outs=[],
                bass_nofuse=True,
                sync_info=bass_rust.SyncInfo(on_update=[], on_wait=[w]),
            )
            _orig_commit_instruction(self, nop, lazy_reg_writes=False)
    return _orig_commit_instruction(self, inst, lazy_reg_writes)


def _split_inst_waits(nc, inst):
    si = inst.ins.sync_info
    if si is None:
        return
    waits = list(si.on_wait)
    if len(waits) <= 1:
        return
    inst.ins.sync_info = bass_rust.SyncInfo(
        on_update=list(si.on_update), on_wait=waits[:1]
    )
    for i in range(1, len(waits)):
        nop = nc.sync.nop(nofuse=True, hint="drain_wait_split")
        nop.ins.sync_info = bass_rust.SyncInfo(on_update=[], on_wait=[waits[i]])


def _patched_drain_and_barrier(self, tick_clock, wait_clock):
    nc = self.nc
    drain_inst = nc.sync.drain()
    wait_clock.add_sem_waits(
        drain_inst.ins, tile_mod.ScopedClock({None: tick_clock.global_clock})
    )
    _split_inst_waits(nc, drain_inst)
    nc.all_engine_barrier()
    assert self.sems is not None
    popped = nc._tile_sem_poison_stack.pop()
    assert popped is self._sem_poison
    nc.clear_and_free_semaphores(list(self.sems.allocated().values()))
    nc.all_engine_barrier()


_orig_commit_instruction = tile_mod.TileContext._commit_instruction
if getattr(tile_mod.TileContext, "_flame_wait_patch", False) is False:
    tile_mod.TileContext._commit_instruction = _patched_commit_instruction
    tile_mod.TileContext._drain_and_barrier = _patched_drain_and_barrier
    tile_mod.TileContext._flame_wait_patch = True

# ----------------------------------------------------------- host-side math


def _rodrigues(r):
    angle = np.linalg.norm(r, axis=-1, keepdims=True) + 1e-8
    axis = r / angle
    x, y, z = axis[..., 0], axis[..., 1], axis[..., 2]
    zero = np.zeros_like(x)
    K = np.stack([zero, -z, y, z, zero, -x, -y, x, zero], axis=-1)
    K = K.reshape(r.shape[:-1] + (3, 3))
    s = np.sin(angle)[..., None]
    c = np.cos(angle)[..., None]
    return np.eye(3, dtype=r.dtype) + s * K + (1.0 - c) * (K @ K)


def _host_batch_prep(shape, expression, rotation, neck, jaw, eyeballs,
                     v_template, shapedirs, J_regressor):
    f64 = np.float64
    b = shape.shape[0]
    betas = np.concatenate([shape, expression], axis=1).astype(f64)
    full_pose = np.concatenate([rotation, neck, jaw, eyeballs], axis=1).astype(f64)

    jr = J_regressor.astype(f64)
    J0 = jr @ v_template.astype(f64)                                   # [5,3]
    SJ = np.einsum('jv,vcl->ljc', jr, shapedirs.astype(f64)).reshape(400, NJ * 3)
    J = (J0.reshape(-1) + betas @ SJ).reshape(b, NJ, 3)

    rot_mats = _rodrigues(full_pose.reshape(b, NJ, 3))
    pose_feature = (rot_mats[:, 1:] - np.eye(3, dtype=f64)).reshape(b, 36)

    rel_joints = np.concatenate([J[:, :1], J[:, 1:] - J[:, PARENTS[1:]]], axis=1)
    T_local = np.zeros((b, NJ, 4, 4), dtype=f64)
    T_local[:, :, :3, :3] = rot_mats
    T_local[:, :, :3, 3] = rel_joints
    T_local[:, :, 3, 3] = 1.0
    chain = [T_local[:, 0]]
    for j in range(1, NJ):
        chain.append(chain[PARENTS[j]] @ T_local[:, j])
    A = np.stack(chain, axis=1)

    j_hom = np.concatenate([J, np.zeros_like(J[..., :1])], axis=-1)
    t_corr = np.einsum('bjmn,bjn->bjm', A, j_hom)
    A_rel = (A - np.concatenate(
        [np.zeros_like(A[..., :3]), t_corr[..., None]], axis=-1)
    ).astype(np.float32)                                               # [B,5,4,4]

    # betas_aug [437, B]: rows 0-399 betas.T, 400 ones, 401-436 pose_feature.T
    betas_aug = np.empty((KB, b), dtype=np.float32)
    betas_aug[:400] = betas.T
    betas_aug[400] = 1.0
    betas_aug[401:] = pose_feature.T
    bt = betas_aug.astype(BF16)

    # artc[c, 32n+j, b] = A_rel[b, j, c, n]; rows at 32-offsets so the
    # K=5 T-blend matmuls can row-tile (tile_position=(32n, 0)).
    artc = np.zeros((3, 128, b), dtype=np.float32)
    for c in range(3):
        for n in range(4):
            for j in range(NJ):
                artc[c, 32 * n + j] = A_rel[:, j, c, n]
    artc = artc.astype(BF16)
    return bt, artc


def _host_model_prep(v_template, shapedirs, posedirs, lbs_weights):
    # sdt [3, 437, VP] matching betas_aug rows, then chunk-major
    # [3, VP//128, 437, 128] so each per-chunk lhsT tile is one
    # contiguous DRAM block (fat DMA descriptors).
    sdt = np.zeros((3, KB, VP), dtype=np.float32)
    sdt[:, :400, :V] = shapedirs.transpose(1, 2, 0)
    sdt[:, 400, :V] = v_template.T
    sdt[:, 401:, :V] = posedirs.reshape(36, V, 3).transpose(2, 0, 1)
    sdt = np.ascontiguousarray(
        sdt.reshape(3, KB, VP // 128, 128).transpose(0, 2, 1, 3)
    ).astype(BF16)

    # wrt [128, VP]: rows 32n+j = w[:, j] for every group n
    wrt = np.zeros((128, VP), dtype=np.float32)
    for n in range(4):
        for j in range(NJ):
            wrt[32 * n + j, :V] = lbs_weights[:, j]
    wrt = wrt.astype(BF16)
    return sdt, wrt

# ------------------------------------------------------------ device kernel


def _build_device_program():
    nc = bass.Bass("TRN2", target_bir_lowering=False, debug=False)
    f32 = mybir.dt.float32
    bf16 = mybir.dt.bfloat16

    sdt = nc.dram_tensor("sdt", [3, NCHUNK, KB, 128], bf16,
                         kind="ExternalInput").ap()
    wrt = nc.dram_tensor("wrt", [128, NVC], bf16, kind="ExternalInput").ap()
    bt = nc.dram_tensor("bt", [KB, B], bf16, kind="ExternalInput").ap()
    artc = nc.dram_tensor("artc", [3, 128, B], bf16, kind="ExternalInput").ap()
    out = nc.dram_tensor("out", [3, NVC, B], bf16, kind="ExternalOutput").ap()

    with tile_mod.TileContext(nc) as tc, ExitStack() as ctx:
        cpool = ctx.enter_context(tc.tile_pool(name="const", bufs=1))
        spool = ctx.enter_context(tc.tile_pool(name="stream", bufs=3))
        vpool = ctx.enter_context(tc.tile_pool(name="vposed", bufs=2))
        tspool = ctx.enter_context(tc.tile_pool(name="tsb", bufs=3))
        apool = ctx.enter_context(tc.tile_pool(name="apply", bufs=3))
        # one shared PSUM pool: 4 bank-tags x bufs=2 = all 8 banks.
        # vposed accumulators round-robin through the tags; each T-round
        # grabs one generation of all 4 tags.
        ps = ctx.enter_context(tc.tile_pool(name="ps", bufs=2, space="PSUM"))
        ps_rr = [0]

        def ps_tile():
            t = ps.tile([128, NS], mybir.dt.float32, tag=f"ps{ps_rr[0]}",
                        name=f"psacc{ps_rr[0]}")
            ps_rr[0] = (ps_rr[0] + 1) % 4
            return t

        def ps_round():
            assert ps_rr[0] == 0
            return [ps.tile([128, NS], mybir.dt.float32, tag=f"ps{n}",
                            name=f"pst{n}")
                    for n in range(4)]

        # resident operands (scalar HWDGE queue so the per-chunk streaming
        # DMAs on the sync queue aren't stuck behind them at startup)
        btt = []
        for ki, (k0, kn) in enumerate(KCH):
            th = cpool.tile([kn, B], bf16, tag=f"bt{ki}", name=f"bt{ki}")
            for cs in range(0, B, NS):
                nc.scalar.dma_start(th[:, cs:cs + NS],
                                    bt[k0:k0 + kn, cs:cs + NS])
            btt.append(th)
        wrtt = cpool.tile([128, NVC], bf16, tag="wrtt")
        nc.scalar.dma_start(wrtt[:], wrt[:, :])
        artt = []
        for c in range(3):
            t = cpool.tile([128, B], bf16, tag=f"artc{c}", name=f"artc{c}")
            nc.scalar.dma_start(t[:], artc[c, :, :])
            artt.append(t)

        for k in range(NCHUNK):
            vs = slice(k * 128, (k + 1) * 128)

            # stream this chunk's vposed lhsT tiles
            st = []
            for c in range(3):
                row = []
                for ki, (k0, kn) in enumerate(KCH):
                    t = spool.tile([kn, 128], bf16, tag=f"st{c}_{ki}",
                                   name=f"st{c}_{ki}")
                    nc.sync.dma_start(t[:], sdt[c, k, k0:k0 + kn, :])
                    row.append(t)
                st.append(row)

            # 1) vposed planes [128, B]: K=437 in 4 chunks, fp32 PSUM,
            #    scalar-copied to SBUF bf16
            nkc = len(KCH)
            vp = []
            for c in range(3):
                dst = vpool.tile([128, B], bf16, tag=f"vp{c}")
                for s in range(B // NS):
                    acc = ps_tile()
                    bs = slice(s * NS, (s + 1) * NS)
                    for ki in range(nkc):
                        nc.tensor.matmul(
                            acc[:], lhsT=st[c][ki][:],
                            rhs=btt[ki][:, bs],
                            start=(ki == 0), stop=(ki == nkc - 1))
                    nc.scalar.copy(out=dst[:, bs], in_=acc[:])
                vp.append(dst)

            # 2+3) per c: T planes + apply over the full batch.
            # Four independent per-plane pipelines, each owning one PSUM
            # bank tag (x2 generations) so chains stay 2 deep.
            def t_mm(dst, c, n, bs):
                p0 = 32 * n
                nc.tensor.matmul(
                    dst[:], lhsT=wrtt[p0:p0 + NJ, vs],
                    rhs=artt[c][p0:p0 + NJ, bs],
                    start=True, stop=True, tile_position=(p0, 0))

            for c in range(3):
                # scalar-copied T planes (bf16 SBUF, full batch)
                t1s = tspool.tile([128, B], bf16, tag="t1s")
                t2s = tspool.tile([128, B], bf16, tag="t2s")
                m0 = apool.tile([128, B], bf16, tag="m0")
                m1 = apool.tile([128, B], bf16, tag="m1")
                m2 = apool.tile([128, B], bf16, tag="m2")
                bpl = apool.tile([128, B], bf16, tag="b")
                av = apool.tile([128, B], bf16, tag="av")
                o = apool.tile([128, B], bf16, tag="o")

                for sub in range(B // NS):
                    bs = slice(sub * NS, (sub + 1) * NS)
                    # T planes: 4-way row-tiled K=5 matmuls, one PSUM
                    # bank each
                    tt = ps_round()
                    for n in range(4):
                        t_mm(tt[n], c, n, bs)
                    # T_c1, T_c2 exit PSUM via scalar (bf16 cast)
                    nc.scalar.copy(out=t1s[:, bs], in_=tt[1][:])
                    nc.scalar.copy(out=t2s[:, bs], in_=tt[2][:])
                    # PSUM-sourced DVE ops at bank granularity
                    nc.vector.tensor_mul(m0[:, bs], tt[0][:], vp[0][:, bs])
                    nc.vector.tensor_mul(m2[:, bs], t2s[:, bs],
                                         vp[2][:, bs])
                    nc.vector.tensor_add(bpl[:, bs], m2[:, bs], tt[3][:])
                # fat SBUF-only ops over the full batch
                nc.vector.tensor_mul(m1[:], t1s[:], vp[1][:])
                nc.vector.tensor_add(av[:], m0[:], m1[:])
                nc.gpsimd.tensor_add(o[:], av[:], bpl[:])
                # single plain output write on the HWDGE sync queue
                nc.sync.dma_start(out[c, vs, :], o[:])
    return nc


_NC_CACHE = {}


def _get_nc():
    if "nc" not in _NC_CACHE:
        _NC_CACHE["nc"] = _build_device_program()
    return _NC_CACHE["nc"]

# ---------------------------------------------------------------- entry


def build_in_maps(shape, expression, rotation, neck, jaw, eyeballs,
                  v_template, shapedirs, posedirs, J_regressor, lbs_weights):
    bt, artc = _host_batch_prep(
        shape, expression, rotation, neck, jaw, eyeballs,
        v_template, shapedirs, J_regressor)
    sdt, wrt = _host_model_prep(v_template, shapedirs, posedirs, lbs_weights)

    in_maps = []
    for i in range(NCORES):
        v0, v1 = i * NVC, (i + 1) * NVC
        in_maps.append({
            "sdt": np.ascontiguousarray(
                sdt[:, i * NCHUNK:(i + 1) * NCHUNK]),
            "wrt": np.ascontiguousarray(wrt[:, v0:v1]),
            "bt": bt,
            "artc": artc,
        })
    return in_maps


def kernel(shape, expression, rotation, neck, jaw, eyeballs,
           v_template, shapedirs, posedirs, J_regressor, lbs_weights):
    in_maps = build_in_maps(shape, expression, rotation, neck, jaw, eyeballs,
                            v_template, shapedirs, posedirs, J_regressor,
                            lbs_weights)
    nc = _get_nc()
    res = run_bass_kernel_spmd(nc, in_maps, core_ids=list(range(NCORES)))

    full = np.concatenate(
        [np.asarray(res.results[i]["out"]) for i in range(NCORES)], axis=1)
    verts = np.ascontiguousarray(
        full[:, :V, :].transpose(2, 1, 0)).astype(np.float32)
    return verts


# revision 18
# speedup vs baseline: 1.0392x; 1.0129x over previous
"""FLAME layer on 8 Trainium2 NeuronCores (Bass/Tile).

Strategy (vertex-parallel, single-pass bf16):
  * V=5023 padded to 5120, 640 vertices/core; every core handles the full
    batch B=2048 for its vertex slice.  Model tensors sliced 1/8 per core,
    per-batch operands (betas, pose features, joint transforms) replicated.
  * Host: rodrigues, FK chain, A_rel; J = J0 + betas @ SJ (linear in betas).
  * Gate is rel_err < 2e-2, so everything runs one bf16 pass (no hi/lo
    splits): ~3x less PE + input DMA than the 3-pass baseline.
  * Device per 128-vertex chunk, [v(part), b(free)] layout:
      1. vposed_c = sdt_c.T @ betas_aug    (PE, K=437, fp32 PSUM)
         -> scalar-copies to SBUF bf16 (vp planes)
      2. T planes (c,n): K=5 matmuls, 4-way row-tiled via tile_position
         -> T_c1/T_c2 scalar-copied to SBUF bf16;
            T_c0/T_c3 consumed straight from PSUM by DVE (1x reads)
      3. apply: m0 = T_c0(PSUM)*vp0 (DVE), m1 = T_c1*vp1 (DVE bf16 2x),
         m2 = T_c2*vp2 (DVE), b = m2 + T_c3(PSUM) (DVE),
         a = m0 + m1 (GpSimd), out = a + b rides the output DMA
         (SWDGE accumulate) in bf16.
      4. host converts bf16 -> f32 and reassembles [B, V, 3].
"""

import os
from contextlib import ExitStack

import ml_dtypes
import numpy as np

import bass_rust
import concourse.bass as bass
import concourse.mybir as mybir
import concourse.tile as tile_mod
from concourse.bass_utils import run_bass_kernel_spmd

# ---------------------------------------------------------------- constants
B = 2048
V = 5023
VP = 5120            # padded vertex count (8 cores x 640)
NVC = VP // 8        # vertices per core
NCHUNK = NVC // 128  # 128-vertex chunks per core (5)
NJ = 5
NCORES = 8
BH = 1024            # batch half (free-dim cell for T/apply)
KB = 437             # vposed contraction rows: 400 betas + 1 ones + 36 pose
KCH = [(k, min(128, KB - k)) for k in range(0, KB, 128)]
NS = 512             # matmul free-dim slice (one PSUM bank)
PARENTS = np.array([-1, 0, 1, 1, 1])

BF16 = ml_dtypes.bfloat16

# ------------------------------------------------- walrus multi-wait patch
# This walrus build accepts only ONE sem-wait per instruction (CTRL and
# LW queue structs alike), but Tile freely assigns several. Split the
# surplus waits onto same-engine NOPs emitted immediately before the
# instruction — the engine stalls on each NOP's wait first, so the
# gating semantics are identical.


def _patched_commit_instruction(self, inst, lazy_reg_writes=True):
    si = inst.sync_info
    if si is not None and len(si.on_wait) > 1:
        waits = list(si.on_wait)
        inst.sync_info = bass_rust.SyncInfo(
            on_update=list(si.on_update), on_wait=waits[:1]
        )
        for w in waits[1:]:
            nop = mybir.InstNoOp(
                name=self.nc.get_next_instruction_name(),
                engine=inst.engine,
                ins=[],
                outs=[],
                bass_nofuse=True,
                sync_info=bass_rust.SyncInfo(on_update=[], on_wait=[w]),
            )
            _orig_commit_instruction(self, nop, lazy_reg_writes=False)
    return _orig_commit_instruction(self, inst, lazy_reg_writes)


def _split_inst_waits(nc, inst):
    si = inst.ins.sync_info
    if si is None:
        return
    waits = list(si.on_wait)
    if len(waits) <= 1:
        return
    inst.ins.sync_info = bass_rust.SyncInfo(
        on_update=list(si.on_update), on_wait=waits[:1]
    )
    for i in range(1, len(waits)):
        nop = nc.sync.nop(nofuse=True, hint="drain_wait_split")
        nop.ins.sync_info = bass_rust.SyncInfo(on_update=[], on_wait=[waits[i]])


def _patched_drain_and_barrier(self, tick_clock, wait_clock):
    nc = self.nc
    drain_inst = nc.sync.drain()
    wait_clock.add_sem_waits(
        drain_inst.ins, tile_mod.ScopedClock({None: tick_clock.global_clock})
    )
    _split_inst_waits(nc, drain_inst)
    nc.all_engine_barrier()
    assert self.sems is not None
    popped = nc._tile_sem_poison_stack.pop()
    assert popped is self._sem_poison
    nc.clear_and_free_semaphores(list(self.sems.allocated().values()))
    nc.all_engine_barrier()


_orig_commit_instruction = tile_mod.TileContext._commit_instruction
if getattr(tile_mod.TileContext, "_flame_wait_patch", False) is False:
    tile_mod.TileContext._commit_instruction = _patched_commit_instruction
    tile_mod.TileContext._drain_and_barrier = _patched_drain_and_barrier
    tile_mod.TileContext._flame_wait_patch = True

# ----------------------------------------------------------- host-side math


def _rodrigues(r):
    angle = np.linalg.norm(r, axis=-1, keepdims=True) + 1e-8
    axis = r / angle
    x, y, z = axis[..., 0], axis[..., 1], axis[..., 2]
    zero = np.zeros_like(x)
    K = np.stack([zero, -z, y, z, zero, -x, -y, x, zero], axis=-1)
    K = K.reshape(r.shape[:-1] + (3, 3))
    s = np.sin(angle)[..., None]
    c = np.cos(angle)[..., None]
    return np.eye(3, dtype=r.dtype) + s * K + (1.0 - c) * (K @ K)


def _host_batch_prep(shape, expression, rotation, neck, jaw, eyeballs,
                     v_template, shapedirs, J_regressor):
    f64 = np.float64
    b = shape.shape[0]
    betas = np.concatenate([shape, expression], axis=1).astype(f64)
    full_pose = np.concatenate([rotation, neck, jaw, eyeballs], axis=1).astype(f64)

    jr = J_regressor.astype(f64)
    J0 = jr @ v_template.astype(f64)                                   # [5,3]
    SJ = np.einsum('jv,vcl->ljc', jr, shapedirs.astype(f64)).reshape(400, NJ * 3)
    J = (J0.reshape(-1) + betas @ SJ).reshape(b, NJ, 3)

    rot_mats = _rodrigues(full_pose.reshape(b, NJ, 3))
    pose_feature = (rot_mats[:, 1:] - np.eye(3, dtype=f64)).reshape(b, 36)

    rel_joints = np.concatenate([J[:, :1], J[:, 1:] - J[:, PARENTS[1:]]], axis=1)
    T_local = np.zeros((b, NJ, 4, 4), dtype=f64)
    T_local[:, :, :3, :3] = rot_mats
    T_local[:, :, :3, 3] = rel_joints
    T_local[:, :, 3, 3] = 1.0
    chain = [T_local[:, 0]]
    for j in range(1, NJ):
        chain.append(chain[PARENTS[j]] @ T_local[:, j])
    A = np.stack(chain, axis=1)

    j_hom = np.concatenate([J, np.zeros_like(J[..., :1])], axis=-1)
    t_corr = np.einsum('bjmn,bjn->bjm', A, j_hom)
    A_rel = (A - np.concatenate(
        [np.zeros_like(A[..., :3]), t_corr[..., None]], axis=-1)
    ).astype(np.float32)                                               # [B,5,4,4]

    # betas_aug [437, B]: rows 0-399 betas.T, 400 ones, 401-436 pose_feature.T
    betas_aug = np.empty((KB, b), dtype=np.float32)
    betas_aug[:400] = betas.T
    betas_aug[400] = 1.0
    betas_aug[401:] = pose_feature.T
    bt = betas_aug.astype(BF16)

    # artc[c, 32n+j, b] = A_rel[b, j, c, n]; rows at 32-offsets so the
    # K=5 T-blend matmuls can row-tile (tile_position=(32n, 0)).
    artc = np.zeros((3, 128, b), dtype=np.float32)
    for c in range(3):
        for n in range(4):
            for j in range(NJ):
                artc[c, 32 * n + j] = A_rel[:, j, c, n]
    artc = artc.astype(BF16)
    return bt, artc


def _host_model_prep(v_template, shapedirs, posedirs, lbs_weights):
    # sdt [3, 437, VP] matching betas_aug rows, then chunk-major
    # [3, VP//128, 437, 128] so each per-chunk lhsT tile is one
    # contiguous DRAM block (fat DMA descriptors).
    sdt = np.zeros((3, KB, VP), dtype=np.float32)
    sdt[:, :400, :V] = shapedirs.transpose(1, 2, 0)
    sdt[:, 400, :V] = v_template.T
    sdt[:, 401:, :V] = posedirs.reshape(36, V, 3).transpose(2, 0, 1)
    sdt = np.ascontiguousarray(
        sdt.reshape(3, KB, VP // 128, 128).transpose(0, 2, 1, 3)
    ).astype(BF16)

    # wrt [128, VP]: rows 32n+j = w[:, j] for every group n
    wrt = np.zeros((128, VP), dtype=np.float32)
    for n in range(4):
        for j in range(NJ):
            wrt[32 * n + j, :V] = lbs_weights[:, j]
    wrt = wrt.astype(BF16)
    return sdt, wrt

# ------------------------------------------------------------ device kernel


def _build_device_program():
    nc = bass.Bass("TRN2", target_bir_lowering=False, debug=False)
    f32 = mybir.dt.float32
    bf16 = mybir.dt.bfloat16

    sdt = nc.dram_tensor("sdt", [3, NCHUNK, KB, 128], bf16,
                         kind="ExternalInput").ap()
    wrt = nc.dram_tensor("wrt", [128, NVC], bf16, kind="ExternalInput").ap()
    bt = nc.dram_tensor("bt", [KB, B], bf16, kind="ExternalInput").ap()
    artc = nc.dram_tensor("artc", [3, 128, B], bf16, kind="ExternalInput").ap()
    out = nc.dram_tensor("out", [3, NVC, B], bf16, kind="ExternalOutput").ap()

    with tile_mod.TileContext(nc) as tc, ExitStack() as ctx:
        cpool = ctx.enter_context(tc.tile_pool(name="const", bufs=1))
        spool = ctx.enter_context(tc.tile_pool(name="stream", bufs=3))
        vpool = ctx.enter_context(tc.tile_pool(name="vposed", bufs=2))
        tspool = ctx.enter_context(tc.tile_pool(name="tsb", bufs=3))
        apool = ctx.enter_context(tc.tile_pool(name="apply", bufs=3))
        # one shared PSUM pool: 4 bank-tags x bufs=2 = all 8 banks.
        # vposed accumulators round-robin through the tags; each T-round
        # grabs one generation of all 4 tags.
        ps = ctx.enter_context(tc.tile_pool(name="ps", bufs=2, space="PSUM"))
        ps_rr = [0]

        def ps_tile():
            t = ps.tile([128, NS], mybir.dt.float32, tag=f"ps{ps_rr[0]}",
                        name=f"psacc{ps_rr[0]}")
            ps_rr[0] = (ps_rr[0] + 1) % 4
            return t

        def ps_round():
            assert ps_rr[0] == 0
            return [ps.tile([128, NS], mybir.dt.float32, tag=f"ps{n}",
                            name=f"pst{n}")
                    for n in range(4)]

        # resident operands (scalar HWDGE queue so the per-chunk streaming
        # DMAs on the sync queue aren't stuck behind them at startup)
        btt = []
        for ki, (k0, kn) in enumerate(KCH):
            th = cpool.tile([kn, B], bf16, tag=f"bt{ki}", name=f"bt{ki}")
            for cs in range(0, B, NS):
                nc.scalar.dma_start(th[:, cs:cs + NS],
                                    bt[k0:k0 + kn, cs:cs + NS])
            btt.append(th)
        wrtt = cpool.tile([128, NVC], bf16, tag="wrtt")
        nc.scalar.dma_start(wrtt[:], wrt[:, :])
        artt = []
        for c in range(3):
            t = cpool.tile([128, B], bf16, tag=f"artc{c}", name=f"artc{c}")
            nc.scalar.dma_start(t[:], artc[c, :, :])
            artt.append(t)

        nkc = len(KCH)

        def stream_sdt(k):
            st = []
            for c in range(3):
                row = []
                for ki, (k0, kn) in enumerate(KCH):
                    t = spool.tile([kn, 128], bf16, tag=f"st{c}_{ki}",
                                   name=f"st{c}_{ki}")
                    nc.sync.dma_start(t[:], sdt[c, k, k0:k0 + kn, :])
                    row.append(t)
                st.append(row)
            return st

        def vp_tiles():
            return [vpool.tile([128, B], bf16, tag=f"vp{c}",
                               name=f"vp{c}") for c in range(3)]

        def emit_vp_group(st, vp, g):
            # one vposed accumulation group: plane c = g // 4,
            # batch-slice s = g % 4
            c, s = g // 4, g % 4
            acc = ps_tile()
            bs = slice(s * NS, (s + 1) * NS)
            for ki in range(nkc):
                nc.tensor.matmul(
                    acc[:], lhsT=st[c][ki][:], rhs=btt[ki][:, bs],
                    start=(ki == 0), stop=(ki == nkc - 1))
            nc.scalar.copy(out=vp[c][:, bs], in_=acc[:])

        def emit_apply_cell(k, c, vp):
            vs = slice(k * 128, (k + 1) * 128)
            t1s = tspool.tile([128, B], bf16, tag="t1s")
            t2s = tspool.tile([128, B], bf16, tag="t2s")
            m0 = apool.tile([128, B], bf16, tag="m0")
            m1 = apool.tile([128, B], bf16, tag="m1")
            m2 = apool.tile([128, B], bf16, tag="m2")
            bpl = apool.tile([128, B], bf16, tag="b")
            av = apool.tile([128, B], bf16, tag="av")
            o = apool.tile([128, B], bf16, tag="o")

            for sub in range(B // NS):
                bs = slice(sub * NS, (sub + 1) * NS)
                # T planes: 4-way row-tiled K=5 matmuls, one PSUM
                # bank each
                tt = ps_round()
                for n in range(4):
                    p0 = 32 * n
                    nc.tensor.matmul(
                        tt[n][:], lhsT=wrtt[p0:p0 + NJ, vs],
                        rhs=artt[c][p0:p0 + NJ, bs],
                        start=True, stop=True, tile_position=(p0, 0))
                # T_c1, T_c2 exit PSUM via scalar (bf16 cast)
                nc.scalar.copy(out=t1s[:, bs], in_=tt[1][:])
                nc.scalar.copy(out=t2s[:, bs], in_=tt[2][:])
                # PSUM-sourced DVE ops at bank granularity
                nc.vector.tensor_mul(m0[:, bs], tt[0][:], vp[0][:, bs])
                nc.vector.tensor_mul(m2[:, bs], t2s[:, bs], vp[2][:, bs])
                nc.vector.tensor_add(bpl[:, bs], m2[:, bs], tt[3][:])
            # fat SBUF-only ops over the full batch
            nc.vector.tensor_mul(m1[:], t1s[:], vp[1][:])
            nc.vector.tensor_add(av[:], m0[:], m1[:])
            nc.gpsimd.tensor_add(o[:], av[:], bpl[:])
            # single plain output write on the HWDGE sync queue
            nc.sync.dma_start(out[c, vs, :], o[:])

        # software pipeline: chunk k's apply cells interleave with
        # chunk k+1's vposed groups so PE/DVE/scalar all stay fed and
        # the PE never idles long enough for HAM to re-throttle.
        st = stream_sdt(0)
        vp_cur = vp_tiles()
        for g in range(12):
            emit_vp_group(st, vp_cur, g)
        for k in range(NCHUNK):
            if k + 1 < NCHUNK:
                st = stream_sdt(k + 1)
                vp_nxt = vp_tiles()
            for c in range(3):
                emit_apply_cell(k, c, vp_cur)
                if k + 1 < NCHUNK:
                    for g in range(4 * c, 4 * c + 4):
                        emit_vp_group(st, vp_nxt, g)
            if k + 1 < NCHUNK:
                vp_cur = vp_nxt
    return nc


_NC_CACHE = {}


def _get_nc():
    if "nc" not in _NC_CACHE:
        _NC_CACHE["nc"] = _build_device_program()
    return _NC_CACHE["nc"]

# ---------------------------------------------------------------- entry


def build_in_maps(shape, expression, rotation, neck, jaw, eyeballs,
                  v_template, shapedirs, posedirs, J_regressor, lbs_weights):
    bt, artc = _host_batch_prep(
        shape, expression, rotation, neck, jaw, eyeballs,
        v_template, shapedirs, J_regressor)
    sdt, wrt = _host_model_prep(v_template, shapedirs, posedirs, lbs_weights)

    in_maps = []
    for i in range(NCORES):
        v0, v1 = i * NVC, (i + 1) * NVC
        in_maps.append({
            "sdt": np.ascontiguousarray(
                sdt[:, i * NCHUNK:(i + 1) * NCHUNK]),
            "wrt": np.ascontiguousarray(wrt[:, v0:v1]),
            "bt": bt,
            "artc": artc,
        })
    return in_maps


def kernel(shape, expression, rotation, neck, jaw, eyeballs,
           v_template, shapedirs, posedirs, J_regressor, lbs_weights):
    in_maps = build_in_maps(shape, expression, rotation, neck, jaw, eyeballs,
                            v_template, shapedirs, posedirs, J_regressor,
                            lbs_weights)
    nc = _get_nc()
    res = run_bass_kernel_spmd(nc, in_maps, core_ids=list(range(NCORES)))

    full = np.concatenate(
        [np.asarray(res.results[i]["out"]) for i in range(NCORES)], axis=1)
    verts = np.ascontiguousarray(
        full[:, :V, :].transpose(2, 1, 0)).astype(np.float32)
    return verts


# revision 20
# speedup vs baseline: 1.1147x; 1.0726x over previous
"""FLAME layer on 8 Trainium2 NeuronCores (Bass/Tile).

Strategy (vertex-parallel, single-pass bf16):
  * V=5023 padded to 5120, 640 vertices/core; every core handles the full
    batch B=2048 for its vertex slice.  Model tensors sliced 1/8 per core,
    per-batch operands (betas, pose features, joint transforms) replicated.
  * Host: rodrigues, FK chain, A_rel; J = J0 + betas @ SJ (linear in betas).
  * Gate is rel_err < 2e-2, so everything runs one bf16 pass (no hi/lo
    splits): ~3x less PE + input DMA than the 3-pass baseline.
  * Device per 128-vertex chunk, [v(part), b(free)] layout:
      1. vposed_c = sdt_c.T @ betas_aug    (PE, K=437, fp32 PSUM)
         -> scalar-copies to SBUF bf16 (vp planes)
      2. T planes (c,n): K=5 matmuls, 4-way row-tiled via tile_position
         -> T_c1/T_c2 scalar-copied to SBUF bf16;
            T_c0/T_c3 consumed straight from PSUM by DVE (1x reads)
      3. apply: m0 = T_c0(PSUM)*vp0 (DVE), m1 = T_c1*vp1 (DVE bf16 2x),
         m2 = T_c2*vp2 (DVE), b = m2 + T_c3(PSUM) (DVE),
         a = m0 + m1 (GpSimd), out = a + b rides the output DMA
         (SWDGE accumulate) in bf16.
      4. host converts bf16 -> f32 and reassembles [B, V, 3].
"""

import os
from contextlib import ExitStack

import ml_dtypes
import numpy as np

import bass_rust
import concourse.bass as bass
import concourse.mybir as mybir
import concourse.tile as tile_mod
from concourse.bass_utils import run_bass_kernel_spmd

# ---------------------------------------------------------------- constants
B = 2048
V = 5023
VP = 5120            # padded vertex count (8 cores x 640)
NVC = VP // 8        # vertices per core
NCHUNK = NVC // 128  # 128-vertex chunks per core (5)
NJ = 5
NCORES = 8
BH = 1024            # batch half (free-dim cell for T/apply)
KB = 437             # vposed contraction rows: 400 betas + 1 ones + 36 pose
KCH = [(k, min(128, KB - k)) for k in range(0, KB, 128)]
NS = 512             # matmul free-dim slice (one PSUM bank)
PARENTS = np.array([-1, 0, 1, 1, 1])

BF16 = ml_dtypes.bfloat16

# ------------------------------------------------- walrus multi-wait patch
# This walrus build accepts only ONE sem-wait per instruction (CTRL and
# LW queue structs alike), but Tile freely assigns several. Split the
# surplus waits onto same-engine NOPs emitted immediately before the
# instruction — the engine stalls on each NOP's wait first, so the
# gating semantics are identical.


def _patched_commit_instruction(self, inst, lazy_reg_writes=True):
    si = inst.sync_info
    if si is not None and len(si.on_wait) > 1:
        waits = list(si.on_wait)
        inst.sync_info = bass_rust.SyncInfo(
            on_update=list(si.on_update), on_wait=waits[:1]
        )
        for w in waits[1:]:
            nop = mybir.InstNoOp(
                name=self.nc.get_next_instruction_name(),
                engine=inst.engine,
                ins=[],
                outs=[],
                bass_nofuse=True,
                sync_info=bass_rust.SyncInfo(on_update=[], on_wait=[w]),
            )
            _orig_commit_instruction(self, nop, lazy_reg_writes=False)
    return _orig_commit_instruction(self, inst, lazy_reg_writes)


def _split_inst_waits(nc, inst):
    si = inst.ins.sync_info
    if si is None:
        return
    waits = list(si.on_wait)
    if len(waits) <= 1:
        return
    inst.ins.sync_info = bass_rust.SyncInfo(
        on_update=list(si.on_update), on_wait=waits[:1]
    )
    for i in range(1, len(waits)):
        nop = nc.sync.nop(nofuse=True, hint="drain_wait_split")
        nop.ins.sync_info = bass_rust.SyncInfo(on_update=[], on_wait=[waits[i]])


def _patched_drain_and_barrier(self, tick_clock, wait_clock):
    nc = self.nc
    drain_inst = nc.sync.drain()
    wait_clock.add_sem_waits(
        drain_inst.ins, tile_mod.ScopedClock({None: tick_clock.global_clock})
    )
    _split_inst_waits(nc, drain_inst)
    nc.all_engine_barrier()
    assert self.sems is not None
    popped = nc._tile_sem_poison_stack.pop()
    assert popped is self._sem_poison
    nc.clear_and_free_semaphores(list(self.sems.allocated().values()))
    nc.all_engine_barrier()


_orig_commit_instruction = tile_mod.TileContext._commit_instruction
if getattr(tile_mod.TileContext, "_flame_wait_patch", False) is False:
    tile_mod.TileContext._commit_instruction = _patched_commit_instruction
    tile_mod.TileContext._drain_and_barrier = _patched_drain_and_barrier
    tile_mod.TileContext._flame_wait_patch = True

# ----------------------------------------------------------- host-side math


def _rodrigues(r):
    angle = np.linalg.norm(r, axis=-1, keepdims=True) + 1e-8
    axis = r / angle
    x, y, z = axis[..., 0], axis[..., 1], axis[..., 2]
    zero = np.zeros_like(x)
    K = np.stack([zero, -z, y, z, zero, -x, -y, x, zero], axis=-1)
    K = K.reshape(r.shape[:-1] + (3, 3))
    s = np.sin(angle)[..., None]
    c = np.cos(angle)[..., None]
    return np.eye(3, dtype=r.dtype) + s * K + (1.0 - c) * (K @ K)


def _host_batch_prep(shape, expression, rotation, neck, jaw, eyeballs,
                     v_template, shapedirs, J_regressor):
    f64 = np.float64
    b = shape.shape[0]
    betas = np.concatenate([shape, expression], axis=1).astype(f64)
    full_pose = np.concatenate([rotation, neck, jaw, eyeballs], axis=1).astype(f64)

    jr = J_regressor.astype(f64)
    J0 = jr @ v_template.astype(f64)                                   # [5,3]
    SJ = np.einsum('jv,vcl->ljc', jr, shapedirs.astype(f64)).reshape(400, NJ * 3)
    J = (J0.reshape(-1) + betas @ SJ).reshape(b, NJ, 3)

    rot_mats = _rodrigues(full_pose.reshape(b, NJ, 3))
    pose_feature = (rot_mats[:, 1:] - np.eye(3, dtype=f64)).reshape(b, 36)

    rel_joints = np.concatenate([J[:, :1], J[:, 1:] - J[:, PARENTS[1:]]], axis=1)
    T_local = np.zeros((b, NJ, 4, 4), dtype=f64)
    T_local[:, :, :3, :3] = rot_mats
    T_local[:, :, :3, 3] = rel_joints
    T_local[:, :, 3, 3] = 1.0
    chain = [T_local[:, 0]]
    for j in range(1, NJ):
        chain.append(chain[PARENTS[j]] @ T_local[:, j])
    A = np.stack(chain, axis=1)

    j_hom = np.concatenate([J, np.zeros_like(J[..., :1])], axis=-1)
    t_corr = np.einsum('bjmn,bjn->bjm', A, j_hom)
    A_rel = (A - np.concatenate(
        [np.zeros_like(A[..., :3]), t_corr[..., None]], axis=-1)
    ).astype(np.float32)                                               # [B,5,4,4]

    # betas_aug [437, B]: rows 0-399 betas.T, 400 ones, 401-436 pose_feature.T
    betas_aug = np.empty((KB, b), dtype=np.float32)
    betas_aug[:400] = betas.T
    betas_aug[400] = 1.0
    betas_aug[401:] = pose_feature.T
    bt = betas_aug.astype(BF16)

    # artc[c, 32n+j, b] = A_rel[b, j, c, n]; rows at 32-offsets so the
    # K=5 T-blend matmuls can row-tile (tile_position=(32n, 0)).
    artc = np.zeros((3, 128, b), dtype=np.float32)
    for c in range(3):
        for n in range(4):
            for j in range(NJ):
                artc[c, 32 * n + j] = A_rel[:, j, c, n]
    artc = artc.astype(BF16)
    return bt, artc


def _host_model_prep(v_template, shapedirs, posedirs, lbs_weights):
    # sdt [3, 437, VP] matching betas_aug rows, then chunk-major
    # [3, VP//128, 437, 128] so each per-chunk lhsT tile is one
    # contiguous DRAM block (fat DMA descriptors).
    sdt = np.zeros((3, KB, VP), dtype=np.float32)
    sdt[:, :400, :V] = shapedirs.transpose(1, 2, 0)
    sdt[:, 400, :V] = v_template.T
    sdt[:, 401:, :V] = posedirs.reshape(36, V, 3).transpose(2, 0, 1)
    sdt = np.ascontiguousarray(
        sdt.reshape(3, KB, VP // 128, 128).transpose(0, 2, 1, 3)
    ).astype(BF16)

    # wrt [128, VP]: rows 32n+j = w[:, j] for every group n
    wrt = np.zeros((128, VP), dtype=np.float32)
    for n in range(4):
        for j in range(NJ):
            wrt[32 * n + j, :V] = lbs_weights[:, j]
    wrt = wrt.astype(BF16)
    return sdt, wrt

# ------------------------------------------------------------ device kernel


def _build_device_program():
    nc = bass.Bass("TRN2", target_bir_lowering=False, debug=False)
    f32 = mybir.dt.float32
    bf16 = mybir.dt.bfloat16

    sdt = nc.dram_tensor("sdt", [3, NCHUNK, KB, 128], bf16,
                         kind="ExternalInput").ap()
    wrt = nc.dram_tensor("wrt", [128, NVC], bf16, kind="ExternalInput").ap()
    bt = nc.dram_tensor("bt", [KB, B], bf16, kind="ExternalInput").ap()
    artc = nc.dram_tensor("artc", [3, 128, B], bf16, kind="ExternalInput").ap()
    out = nc.dram_tensor("out", [3, NVC, B], bf16, kind="ExternalOutput").ap()

    with tile_mod.TileContext(nc) as tc, ExitStack() as ctx:
        cpool = ctx.enter_context(tc.tile_pool(name="const", bufs=1))
        spool = ctx.enter_context(tc.tile_pool(name="stream", bufs=3))
        vpool = ctx.enter_context(tc.tile_pool(name="vposed", bufs=2))
        tspool = ctx.enter_context(tc.tile_pool(name="tsb", bufs=3))
        apool = ctx.enter_context(tc.tile_pool(name="apply", bufs=3))
        # one shared PSUM pool: 4 bank-tags x bufs=2 = all 8 banks.
        # vposed accumulators round-robin through the tags; each T-round
        # grabs one generation of all 4 tags.
        ps = ctx.enter_context(tc.tile_pool(name="ps", bufs=2, space="PSUM"))
        ps_rr = [0]

        def ps_tile():
            t = ps.tile([128, NS], mybir.dt.float32, tag=f"ps{ps_rr[0]}",
                        name=f"psacc{ps_rr[0]}")
            ps_rr[0] = (ps_rr[0] + 1) % 4
            return t

        def ps_round():
            assert ps_rr[0] == 0
            return [ps.tile([128, NS], mybir.dt.float32, tag=f"ps{n}",
                            name=f"pst{n}")
                    for n in range(4)]

        # resident operands (scalar HWDGE queue so the per-chunk streaming
        # DMAs on the sync queue aren't stuck behind them at startup)
        btt = []
        for ki, (k0, kn) in enumerate(KCH):
            th = cpool.tile([kn, B], bf16, tag=f"bt{ki}", name=f"bt{ki}")
            for cs in range(0, B, NS):
                nc.scalar.dma_start(th[:, cs:cs + NS],
                                    bt[k0:k0 + kn, cs:cs + NS])
            btt.append(th)
        wrtt = cpool.tile([128, NVC], bf16, tag="wrtt")
        nc.scalar.dma_start(wrtt[:], wrt[:, :])
        artt = []
        for c in range(3):
            t = cpool.tile([128, B], bf16, tag=f"artc{c}", name=f"artc{c}")
            nc.scalar.dma_start(t[:], artc[c, :, :])
            artt.append(t)

        nkc = len(KCH)

        def stream_sdt(k):
            st = []
            for c in range(3):
                row = []
                for ki, (k0, kn) in enumerate(KCH):
                    t = spool.tile([kn, 128], bf16, tag=f"st{c}_{ki}",
                                   name=f"st{c}_{ki}")
                    nc.sync.dma_start(t[:], sdt[c, k, k0:k0 + kn, :])
                    row.append(t)
                st.append(row)
            return st

        def vp_tiles():
            return [vpool.tile([128, B], bf16, tag=f"vp{c}",
                               name=f"vp{c}") for c in range(3)]

        def emit_vp_group(st, vp, g):
            # one vposed accumulation group: plane c = g // 4,
            # batch-slice s = g % 4
            c, s = g // 4, g % 4
            acc = ps_tile()
            bs = slice(s * NS, (s + 1) * NS)
            for ki in range(nkc):
                nc.tensor.matmul(
                    acc[:], lhsT=st[c][ki][:], rhs=btt[ki][:, bs],
                    start=(ki == 0), stop=(ki == nkc - 1))
            # exits alternate scalar/DVE to balance the two 1x engines
            if g % 2 == 0:
                nc.scalar.copy(out=vp[c][:, bs], in_=acc[:])
            else:
                nc.vector.tensor_copy(vp[c][:, bs], acc[:])

        def emit_apply_cell(k, c, vp):
            vs = slice(k * 128, (k + 1) * 128)
            t1s = tspool.tile([128, B], bf16, tag="t1s")
            t2s = tspool.tile([128, B], bf16, tag="t2s")
            t3s = tspool.tile([128, B], bf16, tag="t3s")
            m0 = apool.tile([128, B], bf16, tag="m0")
            m1 = apool.tile([128, B], bf16, tag="m1")
            m2 = apool.tile([128, B], bf16, tag="m2")
            bpl = apool.tile([128, B], bf16, tag="b")
            av = apool.tile([128, B], bf16, tag="av")

            for sub in range(B // NS):
                bs = slice(sub * NS, (sub + 1) * NS)
                # T planes: 4-way row-tiled K=5 matmuls, one PSUM
                # bank each
                tt = ps_round()
                for n in range(4):
                    p0 = 32 * n
                    nc.tensor.matmul(
                        tt[n][:], lhsT=wrtt[p0:p0 + NJ, vs],
                        rhs=artt[c][p0:p0 + NJ, bs],
                        start=True, stop=True, tile_position=(p0, 0))
                # T_c1, T_c2, T_c3 exit PSUM via scalar (bf16 cast)
                nc.scalar.copy(out=t1s[:, bs], in_=tt[1][:])
                nc.scalar.copy(out=t2s[:, bs], in_=tt[2][:])
                nc.scalar.copy(out=t3s[:, bs], in_=tt[3][:])
                # T_c0 exit fused into its product (DVE, PSUM 1x)
                nc.vector.tensor_mul(m0[:, bs], tt[0][:], vp[0][:, bs])
            # fat SBUF bf16 ops over the full batch (DVE 2x mode)
            nc.vector.tensor_mul(m1[:], t1s[:], vp[1][:])
            nc.vector.tensor_mul(m2[:], t2s[:], vp[2][:])
            nc.vector.tensor_add(bpl[:], m2[:], t3s[:])
            nc.vector.tensor_add(av[:], m0[:], m1[:])
            # final add rides the output DMA (both on the gp SWDGE
            # queue so write -> accum stays ordered)
            nc.gpsimd.dma_start(out[c, vs, :], av[:])
            nc.gpsimd.dma_start(out[c, vs, :], bpl[:],
                                accum_op=mybir.AluOpType.add)

        # software pipeline: chunk k's apply cells interleave with
        # chunk k+1's vposed groups so PE/DVE/scalar all stay fed and
        # the PE never idles long enough for HAM to re-throttle.
        st = stream_sdt(0)
        vp_cur = vp_tiles()
        for g in range(12):
            emit_vp_group(st, vp_cur, g)
        for k in range(NCHUNK):
            if k + 1 < NCHUNK:
                st = stream_sdt(k + 1)
                vp_nxt = vp_tiles()
            for c in range(3):
                emit_apply_cell(k, c, vp_cur)
                if k + 1 < NCHUNK:
                    for g in range(4 * c, 4 * c + 4):
                        emit_vp_group(st, vp_nxt, g)
            if k + 1 < NCHUNK:
                vp_cur = vp_nxt
    return nc


_NC_CACHE = {}


def _get_nc():
    if "nc" not in _NC_CACHE:
        _NC_CACHE["nc"] = _build_device_program()
    return _NC_CACHE["nc"]

# ---------------------------------------------------------------- entry


def build_in_maps(shape, expression, rotation, neck, jaw, eyeballs,
                  v_template, shapedirs, posedirs, J_regressor, lbs_weights):
    bt, artc = _host_batch_prep(
        shape, expression, rotation, neck, jaw, eyeballs,
        v_template, shapedirs, J_regressor)
    sdt, wrt = _host_model_prep(v_template, shapedirs, posedirs, lbs_weights)

    in_maps = []
    for i in range(NCORES):
        v0, v1 = i * NVC, (i + 1) * NVC
        in_maps.append({
            "sdt": np.ascontiguousarray(
                sdt[:, i * NCHUNK:(i + 1) * NCHUNK]),
            "wrt": np.ascontiguousarray(wrt[:, v0:v1]),
            "bt": bt,
            "artc": artc,
        })
    return in_maps


def kernel(shape, expression, rotation, neck, jaw, eyeballs,
           v_template, shapedirs, posedirs, J_regressor, lbs_weights):
    in_maps = build_in_maps(shape, expression, rotation, neck, jaw, eyeballs,
                            v_template, shapedirs, posedirs, J_regressor,
                            lbs_weights)
    nc = _get_nc()
    res = run_bass_kernel_spmd(nc, in_maps, core_ids=list(range(NCORES)))

    full = np.concatenate(
        [np.asarray(res.results[i]["out"]) for i in range(NCORES)], axis=1)
    verts = np.ascontiguousarray(
        full[:, :V, :].transpose(2, 1, 0)).astype(np.float32)
    return verts
